# revision 24
# baseline (speedup 1.0000x reference)
"""Multi-Head Latent Attention (MLA) Trainium2 kernel, 8-way sharded.

Sharding (tensor-parallel heads x data-parallel batch, per the hint):
  core c -> batch b = c // 4, head group hg = c % 4 (4 of 16 heads).
Each core computes the full latent path for its batch (replicated within
the 4-core group), its 4 heads' q_b/kv_b/attention, and a partial o_proj
([D, S] feature-major). Host gathers by summing the 4 partials per batch.

Device dataflow (per core, all activations feature-major [features, S]):
  - x^T supplied by host; projections via matmul(lhsT=W, rhs=act^T)
  - LayerNorm over features (=partitions) via ones-matmul column sums,
    per-token mu/rsigma broadcast back with K=1 fp32r matmuls
  - RoPE via host-prepared rotated/negated weight column copies
    (rotate_half absorbed into q_b / kv_a weights)
  - causal attention with key-major scores^T = k @ q^T, exp without max
    subtraction (scores are small), softmax denominator via ones-matmul
    column sums, normalization deferred to after attn@v
  - compute dtype bf16 with fp32 PSUM accumulation
"""
import numpy as np
import ml_dtypes

import concourse.bass as bass
import concourse.tile as tile
from concourse import bacc, mybir
from concourse.bass_utils import run_bass_kernel_spmd

BF16 = ml_dtypes.bfloat16
F32 = mybir.dt.float32
F32R = mybir.dt.float32r
BF = mybir.dt.bfloat16

D_MODEL = 2048
N_HEADS = 16
Q_LORA = 1536
KV_LORA = 512
NOPE = 128
ROPE = 64
QK_HD = NOPE + ROPE            # 192
V_HD = 128
B, S = 2, 2048
EPS = 1e-5
HPC = 4                        # heads per core
SCALE = float(QK_HD ** -0.5)
NEG = -30000.0                 # additive mask for hidden positions

SC = 512                       # free-dim chunk (matmul N)
NSC = S // SC                  # 4 S-chunks
NKO = D_MODEL // 128           # 16 contraction chunks of x
NQF = Q_LORA // 128            # 12 q-latent feature chunks
KVE = 768                      # kv_a extended out: 512 latent | 64 rope |
                               # 64 pad | 64 rope_rot | 64 pad
NKVF = KVE // 128              # 6
QBO = 1024                     # q_b ext out: 512 nope | 256 rope | 256 rot
NQB = QBO // 128               # 8
NKB = S // 128                 # 16 key blocks
QPB = SC // 128                # 4 key blocks per q chunk


# ---------------------------------------------------------------- host prep

def _rot_cols(W):
    """Columns of W (rope dims, 64) permuted+negated so that
    W_rot.T @ x == rotate_half(W.T @ x)."""
    x1, x2 = W[:, :32], W[:, 32:]
    return np.concatenate([-x2, x1], axis=1)


def _host_prep(x, cos, sin, q_a_w, q_a_ln_w, q_a_ln_b, q_b_w, kv_a_w,
               kv_a_ln_w, kv_a_ln_b, kv_b_w, o_w):
    f32 = np.float32
    x = np.asarray(x, f32); cos = np.asarray(cos, f32); sin = np.asarray(sin, f32)
    q_a_w = np.asarray(q_a_w, f32); q_b_w = np.asarray(q_b_w, f32)
    kv_a_w = np.asarray(kv_a_w, f32); kv_b_w = np.asarray(kv_b_w, f32)
    o_w = np.asarray(o_w, f32)

    # [D, 768] = latent 512 | rope 64 | rope dup 64 | rot 64 | rot dup 64
    # (dup halves so k_rope lands on partitions 0-63 AND 64-127, matching
    #  either base partition of the per-head q_rope operand)
    rope_w = kv_a_w[:, KV_LORA:]
    rot_w = _rot_cols(rope_w)
    kvaw_ext = np.concatenate([kv_a_w, rope_w, rot_w, rot_w], axis=1)

    qb = q_b_w.reshape(Q_LORA, N_HEADS, QK_HD)
    kvb = kv_b_w.reshape(KV_LORA, N_HEADS, NOPE + V_HD)
    ow = o_w.reshape(N_HEADS, V_HD, D_MODEL)

    cos2 = np.concatenate([cos.T, cos.T], axis=0)  # [128, S]
    sin2 = np.concatenate([sin.T, sin.T], axis=0)

    # additive causal masks for diagonal blocks: mask[r, i, c] with
    # visibility c >= 128*i + r
    r = np.arange(128)[:, None, None]
    i = np.arange(QPB)[None, :, None]
    c = np.arange(SC)[None, None, :]
    mask = np.where(c >= 128 * i + r, 0.0, NEG).astype(BF16)  # [128, 4, 512]

    ins = []
    for core in range(8):
        b, hg = divmod(core, 4)
        hs = slice(HPC * hg, HPC * hg + HPC)
        q_nope_w = qb[:, hs, :NOPE].reshape(Q_LORA, HPC * NOPE)
        q_rope_w = qb[:, hs, NOPE:].reshape(Q_LORA, HPC * ROPE)
        q_rot_w = np.concatenate(
            [_rot_cols(qb[:, HPC * hg + h, NOPE:]) for h in range(HPC)], axis=1)
        qbw_ext = np.concatenate([q_nope_w, q_rope_w, q_rot_w], axis=1)
        k_nope_w = kvb[:, hs, :NOPE].reshape(KV_LORA, HPC * NOPE)
        v_w = kvb[:, hs, NOPE:].reshape(KV_LORA, HPC * V_HD)
        kvbw = np.concatenate([k_nope_w, v_w], axis=1)        # [512, 1024]
        ow_c = ow[hs].reshape(HPC * V_HD, D_MODEL)            # [512, 2048]
        ins.append({
            "xT": np.ascontiguousarray(x[b].T).astype(BF16),
            "qaw": q_a_w.astype(BF16),
            "kvaw": kvaw_ext.astype(BF16),
            "qbw": qbw_ext.astype(BF16),
            "kvbw": kvbw.astype(BF16),
            "ow": ow_c.astype(BF16),
            "cos2": cos2.astype(BF16),
            "sin2": sin2.astype(BF16),
            "qlnw": np.ascontiguousarray(
                np.asarray(q_a_ln_w, f32).reshape(NQF, 128).T),
            "qlnb": np.ascontiguousarray(
                np.asarray(q_a_ln_b, f32).reshape(NQF, 128).T),
            "kvlnw": np.ascontiguousarray(
                np.asarray(kv_a_ln_w, f32).reshape(4, 128).T),
            "kvlnb": np.ascontiguousarray(
                np.asarray(kv_a_ln_b, f32).reshape(4, 128).T),
            "mask": mask,
        })
    return ins


# ---------------------------------------------------------------- device IR

def _emit(ctx, tc, T):
    nc = tc.nc
    xT = T["xT"].ap().rearrange("(ko p) s -> p ko s", p=128)      # [128,16,S]
    qaw = T["qaw"].ap().rearrange("(ko p) f -> p ko f", p=128)    # [128,16,1536]
    kvaw = T["kvaw"].ap().rearrange("(ko p) f -> p ko f", p=128)  # [128,16,768]
    qbw = T["qbw"].ap().rearrange("(kc p) m -> p kc m", p=128)    # [128,12,1024]
    kvbw = T["kvbw"].ap().rearrange("(kc p) m -> p kc m", p=128)  # [128,4,1024]
    oww = T["ow"].ap().rearrange("(hc p) d -> p hc d", p=128)     # [128,4,2048]
    outT = T["outT"].ap()                                         # [2048,2048]

    # SBUF budget ~208KB/partition. Slot plan (KB/partition):
    #   big tag A: x S-half [128,16,1024]bf16 32 -> q_all [128,8,2048]bf16 32
    #   big tag B: y_q [128,12,2048]bf16 48     -> ow [128,4,2048]bf16 16
    #   big tag C: y_kv [128,6,2048]bf16 24
    #   big tag D: v [128,16,512]bf16 16
    #   big tag E: att_out [128,4,2048]bf16 16
    #   big tag F: kvbw [128,4,1024]bf16 8
    # big total 144; csts ~13; wstream 8; vec1 8; b512 6; ptp 4; kp 8 => ~191
    csts = ctx.enter_context(tc.tile_pool(name="csts", bufs=1))
    big = ctx.enter_context(tc.tile_pool(name="big", bufs=1))
    wstream = ctx.enter_context(tc.tile_pool(name="wstream", bufs=2))
    vec1 = ctx.enter_context(tc.tile_pool(name="vec1", bufs=4))
    b512 = ctx.enter_context(tc.tile_pool(name="b512", bufs=3))
    ptp = ctx.enter_context(tc.tile_pool(name="ptp", bufs=4))
    kp = ctx.enter_context(tc.tile_pool(name="kp", bufs=2))
    ps_mm = ctx.enter_context(tc.tile_pool(name="ps_mm", bufs=3, space="PSUM"))
    ps_acc = ctx.enter_context(tc.tile_pool(name="ps_acc", bufs=2, space="PSUM"))
    ps_misc = ctx.enter_context(tc.tile_pool(name="ps_misc", bufs=3, space="PSUM"))

    # constants
    ones_bf = csts.tile([128, 1], BF)
    nc.vector.memset(ones_bf[:], 1.0)
    ones_f0 = csts.tile([1, 128], F32)
    nc.vector.memset(ones_f0[:], 1.0)
    ones_f1 = csts.tile([1, 128], F32R)
    nc.scalar.copy(ones_f1[:], ones_f0[:])
    eps_t = csts.tile([1, 1], F32)
    nc.vector.memset(eps_t[:], EPS)
    qlnw = csts.tile([128, NQF], F32); nc.sync.dma_start(qlnw[:], T["qlnw"].ap())
    qlnb = csts.tile([128, NQF], F32); nc.sync.dma_start(qlnb[:], T["qlnb"].ap())
    kvlnw = csts.tile([128, 4], F32); nc.sync.dma_start(kvlnw[:], T["kvlnw"].ap())
    kvlnb = csts.tile([128, 4], F32); nc.sync.dma_start(kvlnb[:], T["kvlnb"].ap())
    mask = csts.tile([128, QPB, SC], BF); nc.sync.dma_start(mask[:], T["mask"].ap())
    cos2 = csts.tile([128, S], BF); nc.sync.dma_start(cos2[:], T["cos2"].ap())
    sin2 = csts.tile([128, S], BF); nc.sync.dma_start(sin2[:], T["sin2"].ap())

    # ---------------- phase A1: latent projections (feature-major),
    # x streamed in S-halves, weights streamed per output chunk (twice)
    y_q = big.tile([128, NQF, S], BF, tag="B")        # q latent pre-LN
    y_kv = big.tile([128, NKVF, S], BF, tag="C")      # kv latent pre-LN + rope
    SH = S // 2
    for half in range(2):
        x_sb = big.tile([128, NKO, SH], BF, tag="A")
        nc.sync.dma_start(x_sb[:, :, :SH // 2],
                          xT[:, :, half * SH:half * SH + SH // 2])
        nc.sync.dma_start(x_sb[:, :, SH // 2:],
                          xT[:, :, half * SH + SH // 2:(half + 1) * SH])
        for dst, wap, nf in ((y_q, qaw, NQF), (y_kv, kvaw, NKVF)):
            for f in range(nf):
                wt = wstream.tile([128, NKO, 128], BF, tag="w")
                nc.sync.dma_start(wt[:], wap[:, :, f * 128:(f + 1) * 128])
                for sch in range(SH // SC):
                    sc = half * (SH // SC) + sch
                    ps = ps_mm.tile([128, SC], F32, tag="mm")
                    for ko in range(NKO):
                        nc.tensor.matmul(ps[:], wt[:, ko, :],
                                         x_sb[:, ko, sch * SC:(sch + 1) * SC],
                                         start=(ko == 0), stop=(ko == NKO - 1))
                    nc.scalar.copy(dst[:, f, sc * SC:(sc + 1) * SC], ps[:])

    # ---------------- phase A2: LN stats + apply, per S-chunk
    def layernorm(y, nf, F, w_pp, b_pp):
        for sc in range(NSC):
            ssl = slice(sc * SC, (sc + 1) * SC)
            s1 = ps_misc.tile([1, SC], F32, tag="misc")
            s2 = ps_misc.tile([1, SC], F32, tag="misc")
            for f in range(nf):
                y2 = b512.tile([128, SC], BF, tag="b512")
                nc.vector.tensor_tensor(y2[:], y[:, f, ssl], y[:, f, ssl],
                                        mybir.AluOpType.mult)
                nc.tensor.matmul(s1[:], ones_bf[:], y[:, f, ssl],
                                 start=(f == 0), stop=(f == nf - 1))
                nc.tensor.matmul(s2[:], ones_bf[:], y2[:],
                                 start=(f == 0), stop=(f == nf - 1))
            mu = vec1.tile([1, SC], F32, tag="vec1")
            rs = vec1.tile([1, SC], F32, tag="vec1")
            var = vec1.tile([1, SC], F32, tag="vec1")
            nc.scalar.activation(mu[:], s1[:],
                                 mybir.ActivationFunctionType.Copy,
                                 scale=1.0 / F)
            nc.scalar.activation(rs[:], s2[:],
                                 mybir.ActivationFunctionType.Copy,
                                 scale=1.0 / F)
            # var = E[y^2] - mu^2 ; rs = 1/sqrt(var + eps)
            nc.vector.tensor_tensor(var[:], mu[:], mu[:], mybir.AluOpType.mult)
            nc.vector.tensor_tensor(var[:], rs[:], var[:],
                                    mybir.AluOpType.subtract)
            nc.scalar.activation(var[:], var[:],
                                 mybir.ActivationFunctionType.Sqrt,
                                 bias=eps_t[:])
            nc.vector.reciprocal(rs[:], var[:])
            # broadcast mu/rs to 128 partitions via K=1 fp32r matmul
            mu_r = vec1.tile([1, SC], F32R, tag="vec1r")
            rs_r = vec1.tile([1, SC], F32R, tag="vec1r")
            nc.scalar.copy(mu_r[:], mu[:])
            nc.scalar.copy(rs_r[:], rs[:])
            mub = ps_misc.tile([128, SC], F32, tag="misc")
            rsb = ps_misc.tile([128, SC], F32, tag="misc")
            nc.tensor.matmul(mub[:], ones_f1[:], mu_r[:], start=True, stop=True)
            nc.tensor.matmul(rsb[:], ones_f1[:], rs_r[:], start=True, stop=True)
            # apply in place: y = (y - mu)*rs*w + b
            for f in range(nf):
                nc.vector.tensor_tensor(y[:, f, ssl], y[:, f, ssl], mub[:],
                                        mybir.AluOpType.subtract)
                nc.vector.tensor_tensor(y[:, f, ssl], y[:, f, ssl], rsb[:],
                                        mybir.AluOpType.mult)
                nc.vector.tensor_scalar(y[:, f, ssl], y[:, f, ssl],
                                        w_pp[:, f:f + 1], b_pp[:, f:f + 1],
                                        mybir.AluOpType.mult,
                                        mybir.AluOpType.add)

    layernorm(y_q, NQF, float(Q_LORA), qlnw, qlnb)
    layernorm(y_kv, 4, float(KV_LORA), kvlnw, kvlnb)

    # ---------------- phase B: q_b projection + RoPE (weights streamed)
    q_all = big.tile([128, NQB, S], BF, tag="A")
    for m in range(NQB):
        wt = wstream.tile([128, NQF, 128], BF, tag="w")
        nc.sync.dma_start(wt[:], qbw[:, :, m * 128:(m + 1) * 128])
        for sc in range(NSC):
            ps = ps_mm.tile([128, SC], F32, tag="mm")
            for kc in range(NQF):
                nc.tensor.matmul(ps[:], wt[:, kc, :],
                                 y_q[:, kc, sc * SC:(sc + 1) * SC],
                                 start=(kc == 0), stop=(kc == NQF - 1))
            nc.scalar.copy(q_all[:, m, sc * SC:(sc + 1) * SC], ps[:])

    # q rope: chunks 4,5 <- chunk{4,5}*cos2 + chunk{6,7}*sin2
    for rc in (4, 5):
        nc.vector.tensor_tensor(q_all[:, rc, :], q_all[:, rc, :], cos2[:],
                                mybir.AluOpType.mult)
        nc.vector.tensor_tensor(q_all[:, rc + 2, :], q_all[:, rc + 2, :],
                                sin2[:], mybir.AluOpType.mult)
        nc.vector.tensor_tensor(q_all[:, rc, :], q_all[:, rc, :],
                                q_all[:, rc + 2, :], mybir.AluOpType.add)
    # k rope: chunk4 = [rope|rope dup], chunk5 = [rot|rot dup] ->
    # chunk4 <- chunk4*cos2 + chunk5*sin2 (both partition halves at once)
    nc.vector.tensor_tensor(y_kv[:, 4, :], y_kv[:, 4, :], cos2[:],
                            mybir.AluOpType.mult)
    nc.vector.tensor_tensor(y_kv[:, 5, :], y_kv[:, 5, :], sin2[:],
                            mybir.AluOpType.mult)
    nc.vector.tensor_tensor(y_kv[:, 4, :], y_kv[:, 4, :],
                            y_kv[:, 5, :], mybir.AluOpType.add)

    # ---------------- phase C: kv_b decompress + attention
    kvbw_sb = big.tile([128, 4, 1024], BF, tag="F")
    nc.sync.dma_start(kvbw_sb[:], kvbw[:])

    # v for all heads, token-major: [128 tok, 16 tchunk, 512(4h x 128)]
    v_sb = big.tile([128, NKB, HPC * V_HD], BF, tag="D")
    for t in range(NKB):
        ps = ps_mm.tile([128, SC], F32, tag="mm")
        for kc in range(4):
            nc.tensor.matmul(ps[:], y_kv[:, kc, t * 128:(t + 1) * 128],
                             kvbw_sb[:, kc, 512:1024],
                             start=(kc == 0), stop=(kc == 3))
        nc.scalar.copy(v_sb[:, t, :], ps[:])

    att_out = big.tile([128, HPC, S], BF, tag="E")
    for h in range(HPC):
        knope = kp.tile([128, S], BF, tag="knope")
        for sc in range(NSC):
            ps = ps_mm.tile([128, SC], F32, tag="mm")
            for kc in range(4):
                nc.tensor.matmul(ps[:], kvbw_sb[:, kc, h * 128:(h + 1) * 128],
                                 y_kv[:, kc, sc * SC:(sc + 1) * SC],
                                 start=(kc == 0), stop=(kc == 3))
            nc.scalar.copy(knope[:, sc * SC:(sc + 1) * SC], ps[:])

        q_nope = q_all[:, h, :]
        rp = (h % 2) * 64
        q_rope = q_all[rp:rp + 64, 4 + h // 2, :]
        for j in range(NSC):
            qsl = slice(j * SC, (j + 1) * SC)
            out_ps = ps_acc.tile([128, SC], F32, tag="av")
            l_ps = ps_misc.tile([1, SC], F32, tag="misc")
            nkb = QPB * (j + 1)
            for kb in range(nkb):
                s_ps = ps_mm.tile([128, SC], F32, tag="mm")
                nc.tensor.matmul(s_ps[:], knope[:, kb * 128:(kb + 1) * 128],
                                 q_nope[:, qsl], start=True, stop=False)
                nc.tensor.matmul(s_ps[:], y_kv[rp:rp + 64, 4,
                                               kb * 128:(kb + 1) * 128],
                                 q_rope[:, qsl], start=False, stop=True)
                pt = ptp.tile([128, SC], BF, tag="pt")
                if kb >= QPB * j:
                    nc.vector.scalar_tensor_tensor(
                        s_ps[:], s_ps[:], SCALE, mask[:, kb - QPB * j, :],
                        mybir.AluOpType.mult, mybir.AluOpType.add)
                    nc.scalar.activation(pt[:], s_ps[:],
                                         mybir.ActivationFunctionType.Exp)
                else:
                    nc.scalar.activation(pt[:], s_ps[:],
                                         mybir.ActivationFunctionType.Exp,
                                         scale=SCALE)
                nc.tensor.matmul(out_ps[:], v_sb[:, kb, h * V_HD:(h + 1) * V_HD],
                                 pt[:], start=(kb == 0), stop=(kb == nkb - 1))
                nc.tensor.matmul(l_ps[:], ones_bf[:], pt[:],
                                 start=(kb == 0), stop=(kb == nkb - 1))
            linv = vec1.tile([1, SC], F32, tag="vec1")
            nc.vector.reciprocal(linv[:], l_ps[:])
            linv_r = vec1.tile([1, SC], F32R, tag="vec1r")
            nc.scalar.copy(linv_r[:], linv[:])
            lbc = ps_misc.tile([128, SC], F32, tag="misc")
            nc.tensor.matmul(lbc[:], ones_f1[:], linv_r[:], start=True, stop=True)
            lbc_sb = b512.tile([128, SC], BF, tag="b512")
            nc.scalar.copy(lbc_sb[:], lbc[:])
            nc.vector.tensor_tensor(att_out[:, h, qsl], out_ps[:], lbc_sb[:],
                                    mybir.AluOpType.mult)

    # ---------------- phase D: o_proj partial, feature-major out
    ow_sb = big.tile([128, HPC, D_MODEL], BF, tag="B")
    nc.sync.dma_start(ow_sb[:], oww[:])
    for dc in range(D_MODEL // 128):
        for sc in range(NSC):
            ps = ps_mm.tile([128, SC], F32, tag="mm")
            for hc in range(HPC):
                nc.tensor.matmul(ps[:], ow_sb[:, hc, dc * 128:(dc + 1) * 128],
                                 att_out[:, hc, sc * SC:(sc + 1) * SC],
                                 start=(hc == 0), stop=(hc == HPC - 1))
            ostage = b512.tile([128, SC], F32, tag="b512")
            nc.scalar.copy(ostage[:], ps[:])
            nc.sync.dma_start(outT[dc * 128:(dc + 1) * 128,
                                   sc * SC:(sc + 1) * SC], ostage[:])


def build_nc(num_devices=8):
    nc = bacc.Bacc("TRN2", target_bir_lowering=False, debug=False,
                   num_devices=num_devices)
    T = {}
    def inp(name, shape, dt):
        T[name] = nc.dram_tensor(name, shape, dt, kind="ExternalInput")
    inp("xT", [D_MODEL, S], BF)
    inp("qaw", [D_MODEL, Q_LORA], BF)
    inp("kvaw", [D_MODEL, KVE], BF)
    inp("qbw", [Q_LORA, QBO], BF)
    inp("kvbw", [KV_LORA, 1024], BF)
    inp("ow", [HPC * V_HD, D_MODEL], BF)
    inp("cos2", [128, S], BF)
    inp("sin2", [128, S], BF)
    inp("qlnw", [128, NQF], F32)
    inp("qlnb", [128, NQF], F32)
    inp("kvlnw", [128, 4], F32)
    inp("kvlnb", [128, 4], F32)
    inp("mask", [128, QPB, SC], BF)
    T["outT"] = nc.dram_tensor("outT", [D_MODEL, S], F32, kind="ExternalOutput")

    from contextlib import ExitStack
    with tile.TileContext(nc) as tc, ExitStack() as ctx:
        _emit(ctx, tc, T)
    nc.compile()
    return nc


_NC_CACHE = {}


def kernel(**inputs) -> np.ndarray:
    in_maps = _host_prep(**inputs)
    if "nc" not in _NC_CACHE:
        _NC_CACHE["nc"] = build_nc()
    nc = _NC_CACHE["nc"]
    res = run_bass_kernel_spmd(nc, in_maps, core_ids=list(range(8)))
    out = np.zeros((B, S, D_MODEL), np.float32)
    for c in range(8):
        out[c // 4] += res.results[c]["outT"].T
    return out


# revision 35
# speedup vs baseline: 1.1652x; 1.1652x over previous
"""Multi-Head Latent Attention (MLA) Trainium2 kernel, 8-way sharded.

Sharding (tensor-parallel heads x data-parallel batch, per the hint):
  core c -> batch b = c // 4, head group hg = c % 4 (4 of 16 heads).
Each core computes the full latent path for its batch (replicated within
the 4-core group), its 4 heads' q_b/kv_b/attention, and a partial o_proj
([D, S] feature-major). Host gathers by summing the 4 partials per batch.

Device dataflow (per core, all activations feature-major [features, S]):
  - x^T supplied by host; projections via matmul(lhsT=W, rhs=act^T)
  - LayerNorm over features (=partitions) via ones-matmul column sums,
    per-token mu/rsigma broadcast back with K=1 fp32r matmuls
  - RoPE via host-prepared rotated/negated weight column copies
    (rotate_half absorbed into q_b / kv_a weights)
  - causal attention with key-major scores^T = k @ q^T, exp without max
    subtraction (scores are small), softmax denominator via ones-matmul
    column sums, normalization deferred to after attn@v
  - compute dtype bf16 with fp32 PSUM accumulation
"""
import numpy as np
import ml_dtypes

import concourse.bass as bass
import concourse.tile as tile
from concourse import bacc, mybir
from concourse.bass_utils import run_bass_kernel_spmd

BF16 = ml_dtypes.bfloat16
F32 = mybir.dt.float32
F32R = mybir.dt.float32r
BF = mybir.dt.bfloat16

D_MODEL = 2048
N_HEADS = 16
Q_LORA = 1536
KV_LORA = 512
NOPE = 128
ROPE = 64
QK_HD = NOPE + ROPE            # 192
V_HD = 128
B, S = 2, 2048
EPS = 1e-5
HPC = 4                        # heads per core
SCALE = float(QK_HD ** -0.5)
NEG = -30000.0                 # additive mask for hidden positions

SC = 512                       # free-dim chunk (matmul N)
NSC = S // SC                  # 4 S-chunks
NKO = D_MODEL // 128           # 16 contraction chunks of x
NQF = Q_LORA // 128            # 12 q-latent feature chunks
KVE = 768                      # kv_a extended out: 512 latent | 64 rope |
                               # 64 pad | 64 rope_rot | 64 pad
NKVF = KVE // 128              # 6
QBO = 1024                     # q_b ext out: 512 nope | 256 rope | 256 rot
NQB = QBO // 128               # 8
NKB = S // 128                 # 16 key blocks
QPB = SC // 128                # 4 key blocks per q chunk


# ---------------------------------------------------------------- host prep

def _rot_cols(W):
    """Columns of W (rope dims, 64) permuted+negated so that
    W_rot.T @ x == rotate_half(W.T @ x)."""
    x1, x2 = W[:, :32], W[:, 32:]
    return np.concatenate([-x2, x1], axis=1)


def _host_prep(x, cos, sin, q_a_w, q_a_ln_w, q_a_ln_b, q_b_w, kv_a_w,
               kv_a_ln_w, kv_a_ln_b, kv_b_w, o_w):
    f32 = np.float32
    x = np.asarray(x, f32); cos = np.asarray(cos, f32); sin = np.asarray(sin, f32)
    q_a_w = np.asarray(q_a_w, f32); q_b_w = np.asarray(q_b_w, f32)
    kv_a_w = np.asarray(kv_a_w, f32); kv_b_w = np.asarray(kv_b_w, f32)
    o_w = np.asarray(o_w, f32)

    # [D, 768] = latent 512 | rope 64 | rope dup 64 | rot 64 | rot dup 64
    # (dup halves so k_rope lands on partitions 0-63 AND 64-127, matching
    #  either base partition of the per-head q_rope operand)
    rope_w = kv_a_w[:, KV_LORA:]
    rot_w = _rot_cols(rope_w)
    kvaw_ext = np.concatenate([kv_a_w, rope_w, rot_w, rot_w], axis=1)

    qb = q_b_w.reshape(Q_LORA, N_HEADS, QK_HD)
    kvb = kv_b_w.reshape(KV_LORA, N_HEADS, NOPE + V_HD)
    ow = o_w.reshape(N_HEADS, V_HD, D_MODEL)

    cos2 = np.concatenate([cos.T, cos.T], axis=0)  # [128, S]
    sin2 = np.concatenate([sin.T, sin.T], axis=0)

    # additive causal masks for diagonal blocks: mask[r, i, c] with
    # visibility c >= 128*i + r
    r = np.arange(128)[:, None, None]
    i = np.arange(QPB)[None, :, None]
    c = np.arange(SC)[None, None, :]
    mask = np.where(c >= 128 * i + r, 0.0, NEG).astype(BF16)  # [128, 4, 512]

    ins = []
    for core in range(8):
        b, hg = divmod(core, 4)
        hs = slice(HPC * hg, HPC * hg + HPC)
        q_nope_w = qb[:, hs, :NOPE].reshape(Q_LORA, HPC * NOPE)
        q_rope_w = qb[:, hs, NOPE:].reshape(Q_LORA, HPC * ROPE)
        q_rot_w = np.concatenate(
            [_rot_cols(qb[:, HPC * hg + h, NOPE:]) for h in range(HPC)], axis=1)
        qbw_ext = np.concatenate([q_nope_w, q_rope_w, q_rot_w], axis=1)
        k_nope_w = kvb[:, hs, :NOPE].reshape(KV_LORA, HPC * NOPE)
        v_w = kvb[:, hs, NOPE:].reshape(KV_LORA, HPC * V_HD)
        kvbw = np.concatenate([k_nope_w, v_w], axis=1)        # [512, 1024]
        ow_c = ow[hs].reshape(HPC * V_HD, D_MODEL)            # [512, 2048]
        ins.append({
            "xT": np.ascontiguousarray(x[b].T).astype(BF16),
            "qaw": q_a_w.astype(BF16),
            "kvaw": kvaw_ext.astype(BF16),
            "qbw": qbw_ext.astype(BF16),
            "kvbw": kvbw.astype(BF16),
            "ow": ow_c.astype(BF16),
            "cos2": cos2.astype(BF16),
            "sin2": sin2.astype(BF16),
            "qlnw": np.ascontiguousarray(
                np.asarray(q_a_ln_w, f32).reshape(NQF, 128).T),
            "qlnb": np.ascontiguousarray(
                np.asarray(q_a_ln_b, f32).reshape(NQF, 128).T),
            "kvlnw": np.ascontiguousarray(
                np.asarray(kv_a_ln_w, f32).reshape(4, 128).T),
            "kvlnb": np.ascontiguousarray(
                np.asarray(kv_a_ln_b, f32).reshape(4, 128).T),
            "mask": mask,
        })
    return ins


# ---------------------------------------------------------------- device IR

def _emit(ctx, tc, T):
    nc = tc.nc
    xT = T["xT"].ap().rearrange("(ko p) s -> p ko s", p=128)      # [128,16,S]
    qaw = T["qaw"].ap().rearrange("(ko p) f -> p ko f", p=128)    # [128,16,1536]
    kvaw = T["kvaw"].ap().rearrange("(ko p) f -> p ko f", p=128)  # [128,16,768]
    qbw = T["qbw"].ap().rearrange("(kc p) m -> p kc m", p=128)    # [128,12,1024]
    kvbw = T["kvbw"].ap().rearrange("(kc p) m -> p kc m", p=128)  # [128,4,1024]
    oww = T["ow"].ap().rearrange("(hc p) d -> p hc d", p=128)     # [128,4,2048]
    outT = T["outT"].ap()                                         # [2048,2048]

    # SBUF budget ~208KB/partition. Slot plan (KB/partition):
    #   big tag A: x S-half [128,16,1024]bf16 32 -> q_all [128,8,2048]bf16 32
    #   big tag B: y_q [128,12,2048]bf16 48     -> ow [128,4,2048]bf16 16
    #   big tag C: y_kv [128,6,2048]bf16 24
    #   big tag D: v [128,16,512]bf16 16
    #   big tag E: att_out [128,4,2048]bf16 16
    #   big tag F: kvbw [128,4,1024]bf16 8
    # big total 144; csts ~13; wstream 8; vec1 8; b512 6; ptp 4; kp 8 => ~191
    csts = ctx.enter_context(tc.tile_pool(name="csts", bufs=1))
    big = ctx.enter_context(tc.tile_pool(name="big", bufs=1))
    wstream = ctx.enter_context(tc.tile_pool(name="wstream", bufs=2))
    vec1 = ctx.enter_context(tc.tile_pool(name="vec1", bufs=4))
    b512 = ctx.enter_context(tc.tile_pool(name="b512", bufs=4))
    ptp = ctx.enter_context(tc.tile_pool(name="ptp", bufs=4))
    kp = ctx.enter_context(tc.tile_pool(name="kp", bufs=2))
    # 8 PSUM banks: ps_mm (5) for projection accumulators / scores /
    # broadcasts; ps_acc (3) for attention out/l accumulators + LN stats
    ps_mm = ctx.enter_context(tc.tile_pool(name="ps_mm", bufs=5, space="PSUM"))
    ps_acc = ctx.enter_context(tc.tile_pool(name="ps_acc", bufs=3, space="PSUM"))

    # constants
    ones_bf = csts.tile([128, 1], BF)
    nc.vector.memset(ones_bf[:], 1.0)
    ones_f0 = csts.tile([1, 128], F32)
    nc.vector.memset(ones_f0[:], 1.0)
    ones_f1 = csts.tile([1, 128], F32R)
    nc.scalar.copy(ones_f1[:], ones_f0[:])
    eps128 = csts.tile([128, 1], F32)
    nc.vector.memset(eps128[:], EPS)
    qlnw = csts.tile([128, NQF], F32); nc.sync.dma_start(qlnw[:], T["qlnw"].ap())
    qlnb = csts.tile([128, NQF], F32); nc.sync.dma_start(qlnb[:], T["qlnb"].ap())
    kvlnw = csts.tile([128, 4], F32); nc.sync.dma_start(kvlnw[:], T["kvlnw"].ap())
    kvlnb = csts.tile([128, 4], F32); nc.sync.dma_start(kvlnb[:], T["kvlnb"].ap())
    mask = csts.tile([128, QPB, SC], BF); nc.sync.dma_start(mask[:], T["mask"].ap())
    cos2 = csts.tile([128, S], BF); nc.sync.dma_start(cos2[:], T["cos2"].ap())
    sin2 = csts.tile([128, S], BF); nc.sync.dma_start(sin2[:], T["sin2"].ap())

    # ---------------- phase A1: latent projections (feature-major),
    # x streamed in S-halves, weights streamed per output chunk (twice)
    y_q = big.tile([128, NQF, S], BF, tag="B")        # q latent pre-LN
    y_kv = big.tile([128, NKVF, S], BF, tag="C")      # kv latent pre-LN + rope
    SH = S // 2
    for half in range(2):
        x_sb = big.tile([128, NKO, SH], BF, tag="A")
        nc.sync.dma_start(x_sb[:, :, :SH // 2],
                          xT[:, :, half * SH:half * SH + SH // 2])
        nc.sync.dma_start(x_sb[:, :, SH // 2:],
                          xT[:, :, half * SH + SH // 2:(half + 1) * SH])
        for dst, wap, nf in ((y_q, qaw, NQF), (y_kv, kvaw, NKVF)):
            for f in range(nf):
                wt = wstream.tile([128, NKO, 128], BF, tag="w")
                nc.sync.dma_start(wt[:], wap[:, :, f * 128:(f + 1) * 128])
                nsch = SH // SC
                pss = [ps_mm.tile([128, SC], F32, tag="mm", name=f"psA{i}")
                       for i in range(nsch)]
                for ko in range(NKO):  # weight-stationary over S-chunks
                    for sch in range(nsch):
                        nc.tensor.matmul(pss[sch][:], wt[:, ko, :],
                                         x_sb[:, ko, sch * SC:(sch + 1) * SC],
                                         start=(ko == 0), stop=(ko == NKO - 1))
                for sch in range(nsch):
                    sc = half * nsch + sch
                    nc.scalar.copy(dst[:, f, sc * SC:(sc + 1) * SC],
                                   pss[sch][:])

    # ---------------- phase A2: LN stats + apply, per S-chunk
    def layernorm(y, nf, F, w_pp, b_pp):
        for sc in range(NSC):
            ssl = slice(sc * SC, (sc + 1) * SC)
            s1 = ps_acc.tile([1, SC], F32, tag="accl")
            s2 = ps_acc.tile([1, SC], F32, tag="accl")
            for f in range(nf):
                y2 = b512.tile([128, SC], BF, tag="b512")
                nc.vector.tensor_tensor(y2[:], y[:, f, ssl], y[:, f, ssl],
                                        mybir.AluOpType.mult)
                nc.tensor.matmul(s1[:], ones_bf[:], y[:, f, ssl],
                                 start=(f == 0), stop=(f == nf - 1))
                nc.tensor.matmul(s2[:], ones_bf[:], y2[:],
                                 start=(f == 0), stop=(f == nf - 1))
            # E[y] and E[y^2] as fp32r rows, broadcast wide via K=1 matmul
            mu_r = vec1.tile([1, SC], F32R, tag="vec1r")
            ms_r = vec1.tile([1, SC], F32R, tag="vec1r")
            nc.scalar.activation(mu_r[:], s1[:],
                                 mybir.ActivationFunctionType.Copy,
                                 scale=1.0 / F)
            nc.scalar.activation(ms_r[:], s2[:],
                                 mybir.ActivationFunctionType.Copy,
                                 scale=1.0 / F)
            mub_ps = ps_mm.tile([128, SC], F32, tag="mm")
            msb_ps = ps_mm.tile([128, SC], F32, tag="mm")
            nc.tensor.matmul(mub_ps[:], ones_f1[:], mu_r[:],
                             start=True, stop=True)
            nc.tensor.matmul(msb_ps[:], ones_f1[:], ms_r[:],
                             start=True, stop=True)
            # wide finalize: rs = 1/sqrt(E[y^2] - mu^2 + eps) on 128 lanes
            mub_sb = b512.tile([128, SC], F32, tag="b512")
            nc.scalar.copy(mub_sb[:], mub_ps[:])
            var_sb = b512.tile([128, SC], F32, tag="b512")
            nc.vector.tensor_tensor(var_sb[:], mub_sb[:], mub_sb[:],
                                    mybir.AluOpType.mult)
            nc.vector.tensor_tensor(var_sb[:], msb_ps[:], var_sb[:],
                                    mybir.AluOpType.subtract)
            nc.scalar.activation(var_sb[:], var_sb[:],
                                 mybir.ActivationFunctionType.Sqrt,
                                 bias=eps128[:])
            rsb_sb = b512.tile([128, SC], F32, tag="b512")
            nc.vector.reciprocal(rsb_sb[:], var_sb[:])
            # apply in place: y = (y - mu)*rs*w + b
            for f in range(nf):
                nc.vector.tensor_tensor(y[:, f, ssl], y[:, f, ssl], mub_sb[:],
                                        mybir.AluOpType.subtract)
                nc.vector.tensor_tensor(y[:, f, ssl], y[:, f, ssl], rsb_sb[:],
                                        mybir.AluOpType.mult)
                nc.vector.tensor_scalar(y[:, f, ssl], y[:, f, ssl],
                                        w_pp[:, f:f + 1], b_pp[:, f:f + 1],
                                        mybir.AluOpType.mult,
                                        mybir.AluOpType.add)

    layernorm(y_q, NQF, float(Q_LORA), qlnw, qlnb)
    layernorm(y_kv, 4, float(KV_LORA), kvlnw, kvlnb)

    # ---------------- phase B: q_b projection + RoPE (weights streamed)
    q_all = big.tile([128, NQB, S], BF, tag="A")
    for m in range(NQB):
        wt = wstream.tile([128, NQF, 128], BF, tag="w")
        nc.sync.dma_start(wt[:], qbw[:, :, m * 128:(m + 1) * 128])
        pss = [ps_mm.tile([128, SC], F32, tag="mm", name=f"psB{i}") for i in range(NSC)]
        for kc in range(NQF):  # weight-stationary over S-chunks
            for sc in range(NSC):
                nc.tensor.matmul(pss[sc][:], wt[:, kc, :],
                                 y_q[:, kc, sc * SC:(sc + 1) * SC],
                                 start=(kc == 0), stop=(kc == NQF - 1))
        for sc in range(NSC):
            nc.scalar.copy(q_all[:, m, sc * SC:(sc + 1) * SC], pss[sc][:])

    # q rope: chunks 4,5 <- chunk{4,5}*cos2 + chunk{6,7}*sin2
    for rc in (4, 5):
        nc.vector.tensor_tensor(q_all[:, rc, :], q_all[:, rc, :], cos2[:],
                                mybir.AluOpType.mult)
        nc.vector.tensor_tensor(q_all[:, rc + 2, :], q_all[:, rc + 2, :],
                                sin2[:], mybir.AluOpType.mult)
        nc.vector.tensor_tensor(q_all[:, rc, :], q_all[:, rc, :],
                                q_all[:, rc + 2, :], mybir.AluOpType.add)
    # k rope: chunk4 = [rope|rope dup], chunk5 = [rot|rot dup] ->
    # chunk4 <- chunk4*cos2 + chunk5*sin2 (both partition halves at once)
    nc.vector.tensor_tensor(y_kv[:, 4, :], y_kv[:, 4, :], cos2[:],
                            mybir.AluOpType.mult)
    nc.vector.tensor_tensor(y_kv[:, 5, :], y_kv[:, 5, :], sin2[:],
                            mybir.AluOpType.mult)
    nc.vector.tensor_tensor(y_kv[:, 4, :], y_kv[:, 4, :],
                            y_kv[:, 5, :], mybir.AluOpType.add)

    # ---------------- phase C: kv_b decompress + attention
    kvbw_sb = big.tile([128, 4, 1024], BF, tag="F")
    nc.sync.dma_start(kvbw_sb[:], kvbw[:])

    # v for all heads, token-major: [128 tok, 16 tchunk, 512(4h x 128)]
    v_sb = big.tile([128, NKB, HPC * V_HD], BF, tag="D")
    for t in range(NKB):
        ps = ps_mm.tile([128, SC], F32, tag="mm")
        for kc in range(4):
            nc.tensor.matmul(ps[:], y_kv[:, kc, t * 128:(t + 1) * 128],
                             kvbw_sb[:, kc, 512:1024],
                             start=(kc == 0), stop=(kc == 3))
        nc.scalar.copy(v_sb[:, t, :], ps[:])

    att_out = big.tile([128, HPC, S], BF, tag="E")
    for h in range(HPC):
        knope = kp.tile([128, S], BF, tag="knope")
        pss = [ps_mm.tile([128, SC], F32, tag="mm", name=f"psK{i}") for i in range(NSC)]
        for kc in range(4):  # weight-stationary over S-chunks
            for sc in range(NSC):
                nc.tensor.matmul(pss[sc][:], kvbw_sb[:, kc, h * 128:(h + 1) * 128],
                                 y_kv[:, kc, sc * SC:(sc + 1) * SC],
                                 start=(kc == 0), stop=(kc == 3))
        for sc in range(NSC):
            nc.scalar.copy(knope[:, sc * SC:(sc + 1) * SC], pss[sc][:])

        q_nope = q_all[:, h, :]
        rp = (h % 2) * 64
        q_rope = q_all[rp:rp + 64, 4 + h // 2, :]
        for j in range(NSC):
            qsl = slice(j * SC, (j + 1) * SC)
            out_ps = ps_acc.tile([128, SC], F32, tag="accl")
            l_ps = ps_acc.tile([1, SC], F32, tag="accl")
            nkb = QPB * (j + 1)
            for kb in range(nkb):
                s_ps = ps_mm.tile([128, SC], F32, tag="mm")
                nc.tensor.matmul(s_ps[:], knope[:, kb * 128:(kb + 1) * 128],
                                 q_nope[:, qsl], start=True, stop=False)
                nc.tensor.matmul(s_ps[:], y_kv[rp:rp + 64, 4,
                                               kb * 128:(kb + 1) * 128],
                                 q_rope[:, qsl], start=False, stop=True)
                pt = ptp.tile([128, SC], BF, tag="pt")
                if kb >= QPB * j:
                    nc.vector.scalar_tensor_tensor(
                        s_ps[:], s_ps[:], SCALE, mask[:, kb - QPB * j, :],
                        mybir.AluOpType.mult, mybir.AluOpType.add)
                    nc.scalar.activation(pt[:], s_ps[:],
                                         mybir.ActivationFunctionType.Exp)
                else:
                    nc.scalar.activation(pt[:], s_ps[:],
                                         mybir.ActivationFunctionType.Exp,
                                         scale=SCALE)
                nc.tensor.matmul(out_ps[:], v_sb[:, kb, h * V_HD:(h + 1) * V_HD],
                                 pt[:], start=(kb == 0), stop=(kb == nkb - 1))
                nc.tensor.matmul(l_ps[:], ones_bf[:], pt[:],
                                 start=(kb == 0), stop=(kb == nkb - 1))
            l_r = vec1.tile([1, SC], F32R, tag="vec1r")
            nc.scalar.copy(l_r[:], l_ps[:])
            lbc = ps_mm.tile([128, SC], F32, tag="mm")
            nc.tensor.matmul(lbc[:], ones_f1[:], l_r[:], start=True, stop=True)
            linvb = b512.tile([128, SC], BF, tag="b512")
            with nc.allow_low_precision(reason="1/l softmax denom in bf16"):
                nc.vector.reciprocal(linvb[:], lbc[:])
            nc.vector.tensor_tensor(att_out[:, h, qsl], out_ps[:], linvb[:],
                                    mybir.AluOpType.mult)

    # ---------------- phase D: o_proj partial, feature-major out
    ow_sb = big.tile([128, HPC, D_MODEL], BF, tag="B")
    nc.sync.dma_start(ow_sb[:], oww[:])
    for dc in range(D_MODEL // 128):
        pss = [ps_mm.tile([128, SC], F32, tag="mm", name=f"psD{i}") for i in range(NSC)]
        for hc in range(HPC):  # weight-stationary over S-chunks
            for sc in range(NSC):
                nc.tensor.matmul(pss[sc][:],
                                 ow_sb[:, hc, dc * 128:(dc + 1) * 128],
                                 att_out[:, hc, sc * SC:(sc + 1) * SC],
                                 start=(hc == 0), stop=(hc == HPC - 1))
        for sc in range(NSC):
            ostage = b512.tile([128, SC], F32, tag="b512")
            nc.scalar.copy(ostage[:], pss[sc][:])
            nc.sync.dma_start(outT[dc * 128:(dc + 1) * 128,
                                   sc * SC:(sc + 1) * SC], ostage[:])


def build_nc(num_devices=8):
    nc = bacc.Bacc("TRN2", target_bir_lowering=False, debug=False,
                   num_devices=num_devices)
    T = {}
    def inp(name, shape, dt):
        T[name] = nc.dram_tensor(name, shape, dt, kind="ExternalInput")
    inp("xT", [D_MODEL, S], BF)
    inp("qaw", [D_MODEL, Q_LORA], BF)
    inp("kvaw", [D_MODEL, KVE], BF)
    inp("qbw", [Q_LORA, QBO], BF)
    inp("kvbw", [KV_LORA, 1024], BF)
    inp("ow", [HPC * V_HD, D_MODEL], BF)
    inp("cos2", [128, S], BF)
    inp("sin2", [128, S], BF)
    inp("qlnw", [128, NQF], F32)
    inp("qlnb", [128, NQF], F32)
    inp("kvlnw", [128, 4], F32)
    inp("kvlnb", [128, 4], F32)
    inp("mask", [128, QPB, SC], BF)
    T["outT"] = nc.dram_tensor("outT", [D_MODEL, S], F32, kind="ExternalOutput")

    from contextlib import ExitStack
    with tile.TileContext(nc) as tc, ExitStack() as ctx:
        _emit(ctx, tc, T)
    nc.compile()
    return nc


_NC_CACHE = {}


def kernel(**inputs) -> np.ndarray:
    in_maps = _host_prep(**inputs)
    if "nc" not in _NC_CACHE:
        _NC_CACHE["nc"] = build_nc()
    nc = _NC_CACHE["nc"]
    res = run_bass_kernel_spmd(nc, in_maps, core_ids=list(range(8)))
    out = np.zeros((B, S, D_MODEL), np.float32)
    for c in range(8):
        out[c // 4] += res.results[c]["outT"].T
    return out


# revision 39
# speedup vs baseline: 1.2785x; 1.0973x over previous
"""Multi-Head Latent Attention (MLA) Trainium2 kernel, 8-way sharded.

Sharding (tensor-parallel heads x data-parallel batch, per the hint):
  core c -> batch b = c // 4, head group hg = c % 4 (4 of 16 heads).
Each core computes the full latent path for its batch (replicated within
the 4-core group), its 4 heads' q_b/kv_b/attention, and a partial o_proj
([D, S] feature-major). Host gathers by summing the 4 partials per batch.

Device dataflow (per core, all activations feature-major [features, S]):
  - x^T supplied by host; projections via matmul(lhsT=W, rhs=act^T)
  - LayerNorm over features (=partitions) via ones-matmul column sums,
    per-token mu/rsigma broadcast back with K=1 fp32r matmuls
  - RoPE via host-prepared rotated/negated weight column copies
    (rotate_half absorbed into q_b / kv_a weights)
  - causal attention with key-major scores^T = k @ q^T, exp without max
    subtraction (scores are small), softmax denominator via ones-matmul
    column sums, normalization deferred to after attn@v
  - compute dtype bf16 with fp32 PSUM accumulation
"""
import numpy as np
import ml_dtypes

import concourse.bass as bass
import concourse.tile as tile
from concourse import bacc, mybir
from concourse.bass_utils import run_bass_kernel_spmd

BF16 = ml_dtypes.bfloat16
F32 = mybir.dt.float32
F32R = mybir.dt.float32r
BF = mybir.dt.bfloat16

D_MODEL = 2048
N_HEADS = 16
Q_LORA = 1536
KV_LORA = 512
NOPE = 128
ROPE = 64
QK_HD = NOPE + ROPE            # 192
V_HD = 128
B, S = 2, 2048
EPS = 1e-5
HPC = 4                        # heads per core
SCALE = float(QK_HD ** -0.5)
NEG = -30000.0                 # additive mask for hidden positions

SC = 512                       # free-dim chunk (matmul N)
NSC = S // SC                  # 4 S-chunks
NKO = D_MODEL // 128           # 16 contraction chunks of x
NQF = Q_LORA // 128            # 12 q-latent feature chunks
KVE = 768                      # kv_a extended out: 512 latent | 64 rope |
                               # 64 pad | 64 rope_rot | 64 pad
NKVF = KVE // 128              # 6
QBO = 1024                     # q_b ext out: 512 nope | 256 rope | 256 rot
NQB = QBO // 128               # 8
NKB = S // 128                 # 16 key blocks
QPB = SC // 128                # 4 key blocks per q chunk


# ---------------------------------------------------------------- host prep

def _rot_cols(W):
    """Columns of W (rope dims, 64) permuted+negated so that
    W_rot.T @ x == rotate_half(W.T @ x)."""
    x1, x2 = W[:, :32], W[:, 32:]
    return np.concatenate([-x2, x1], axis=1)


def _host_prep(x, cos, sin, q_a_w, q_a_ln_w, q_a_ln_b, q_b_w, kv_a_w,
               kv_a_ln_w, kv_a_ln_b, kv_b_w, o_w):
    f32 = np.float32
    x = np.asarray(x, f32); cos = np.asarray(cos, f32); sin = np.asarray(sin, f32)
    q_a_w = np.asarray(q_a_w, f32); q_b_w = np.asarray(q_b_w, f32)
    kv_a_w = np.asarray(kv_a_w, f32); kv_b_w = np.asarray(kv_b_w, f32)
    o_w = np.asarray(o_w, f32)

    # [D, 768] = latent 512 | rope 64 | rope dup 64 | rot 64 | rot dup 64
    # (dup halves so k_rope lands on partitions 0-63 AND 64-127, matching
    #  either base partition of the per-head q_rope operand)
    rope_w = kv_a_w[:, KV_LORA:]
    rot_w = _rot_cols(rope_w)
    kvaw_ext = np.concatenate([kv_a_w, rope_w, rot_w, rot_w], axis=1)

    qb = q_b_w.reshape(Q_LORA, N_HEADS, QK_HD)
    kvb = kv_b_w.reshape(KV_LORA, N_HEADS, NOPE + V_HD)
    ow = o_w.reshape(N_HEADS, V_HD, D_MODEL)

    cos2 = np.concatenate([cos.T, cos.T], axis=0)  # [128, S]
    sin2 = np.concatenate([sin.T, sin.T], axis=0)

    # additive causal masks for diagonal blocks: mask[r, i, c] with
    # visibility c >= 128*i + r
    r = np.arange(128)[:, None, None]
    i = np.arange(QPB)[None, :, None]
    c = np.arange(SC)[None, None, :]
    mask = np.where(c >= 128 * i + r, 0.0, NEG).astype(BF16)  # [128, 4, 512]

    ins = []
    for core in range(8):
        b, hg = divmod(core, 4)
        hs = slice(HPC * hg, HPC * hg + HPC)
        q_nope_w = qb[:, hs, :NOPE].reshape(Q_LORA, HPC * NOPE)
        q_rope_w = qb[:, hs, NOPE:].reshape(Q_LORA, HPC * ROPE)
        q_rot_w = np.concatenate(
            [_rot_cols(qb[:, HPC * hg + h, NOPE:]) for h in range(HPC)], axis=1)
        qbw_ext = np.concatenate([q_nope_w, q_rope_w, q_rot_w], axis=1)
        k_nope_w = kvb[:, hs, :NOPE].reshape(KV_LORA, HPC * NOPE)
        v_w = kvb[:, hs, NOPE:].reshape(KV_LORA, HPC * V_HD)
        kvbw = np.concatenate([k_nope_w, v_w], axis=1)        # [512, 1024]
        ow_c = ow[hs].reshape(HPC * V_HD, D_MODEL)            # [512, 2048]
        ins.append({
            "xT": np.ascontiguousarray(x[b].T).astype(BF16),
            "qaw": q_a_w.astype(BF16),
            "kvaw": kvaw_ext.astype(BF16),
            "qbw": qbw_ext.astype(BF16),
            "kvbw": kvbw.astype(BF16),
            "ow": ow_c.astype(BF16),
            "cos2": cos2.astype(BF16),
            "sin2": sin2.astype(BF16),
            "qlnw": np.ascontiguousarray(
                np.asarray(q_a_ln_w, f32).reshape(NQF, 128).T),
            "qlnb": np.ascontiguousarray(
                np.asarray(q_a_ln_b, f32).reshape(NQF, 128).T),
            "kvlnw": np.ascontiguousarray(
                np.asarray(kv_a_ln_w, f32).reshape(4, 128).T),
            "kvlnb": np.ascontiguousarray(
                np.asarray(kv_a_ln_b, f32).reshape(4, 128).T),
            "mask": mask,
        })
    return ins


# ---------------------------------------------------------------- device IR

def _emit(ctx, tc, T):
    nc = tc.nc
    xT = T["xT"].ap().rearrange("(ko p) s -> p ko s", p=128)      # [128,16,S]
    qaw = T["qaw"].ap().rearrange("(ko p) f -> p ko f", p=128)    # [128,16,1536]
    kvaw = T["kvaw"].ap().rearrange("(ko p) f -> p ko f", p=128)  # [128,16,768]
    qbw = T["qbw"].ap().rearrange("(kc p) m -> p kc m", p=128)    # [128,12,1024]
    kvbw = T["kvbw"].ap().rearrange("(kc p) m -> p kc m", p=128)  # [128,4,1024]
    oww = T["ow"].ap().rearrange("(hc p) d -> p hc d", p=128)     # [128,4,2048]
    outT = T["outT"].ap()                                         # [2048,2048]

    # SBUF budget ~208KB/partition. Slot plan (KB/partition):
    #   big tag A: x S-half [128,16,1024]bf16 32 -> q_all [128,8,2048]bf16 32
    #   big tag B: y_q [128,12,2048]bf16 48     -> ow [128,4,2048]bf16 16
    #   big tag C: y_kv [128,6,2048]bf16 24
    #   big tag D: v [128,16,512]bf16 16
    #   big tag E: att_out [128,4,2048]bf16 16
    #   big tag F: kvbw [128,4,1024]bf16 8
    # big total 144; csts ~13; wstream 8; vec1 8; b512 6; ptp 4; kp 8 => ~191
    csts = ctx.enter_context(tc.tile_pool(name="csts", bufs=1))
    big = ctx.enter_context(tc.tile_pool(name="big", bufs=1))
    wstream = ctx.enter_context(tc.tile_pool(name="wstream", bufs=2))
    vec1 = ctx.enter_context(tc.tile_pool(name="vec1", bufs=4))
    b512 = ctx.enter_context(tc.tile_pool(name="b512", bufs=4))
    ptp = ctx.enter_context(tc.tile_pool(name="ptp", bufs=4))
    kp = ctx.enter_context(tc.tile_pool(name="kp", bufs=2))
    # 8 PSUM banks: ps_mm (5) for projection accumulators / scores /
    # broadcasts; ps_acc (3) for attention out/l accumulators + LN stats
    ps_mm = ctx.enter_context(tc.tile_pool(name="ps_mm", bufs=5, space="PSUM"))
    ps_acc = ctx.enter_context(tc.tile_pool(name="ps_acc", bufs=3, space="PSUM"))

    # constants
    ones_bf = csts.tile([128, 1], BF)
    nc.vector.memset(ones_bf[:], 1.0)
    ones_f0 = csts.tile([1, 128], F32)
    nc.vector.memset(ones_f0[:], 1.0)
    ones_f1 = csts.tile([1, 128], F32R)
    nc.scalar.copy(ones_f1[:], ones_f0[:])
    eps128 = csts.tile([128, 1], F32)
    nc.vector.memset(eps128[:], EPS)
    qlnw = csts.tile([128, NQF], F32); nc.sync.dma_start(qlnw[:], T["qlnw"].ap())
    qlnb = csts.tile([128, NQF], F32); nc.sync.dma_start(qlnb[:], T["qlnb"].ap())
    kvlnw = csts.tile([128, 4], F32); nc.sync.dma_start(kvlnw[:], T["kvlnw"].ap())
    kvlnb = csts.tile([128, 4], F32); nc.sync.dma_start(kvlnb[:], T["kvlnb"].ap())
    mask = csts.tile([128, QPB, SC], BF); nc.sync.dma_start(mask[:], T["mask"].ap())
    cos2 = csts.tile([128, S], BF); nc.sync.dma_start(cos2[:], T["cos2"].ap())
    sin2 = csts.tile([128, S], BF); nc.sync.dma_start(sin2[:], T["sin2"].ap())

    # ---------------- phase A: latent projections (feature-major) with
    # LayerNorm pipelined per S-half: x streamed in S-halves, weights
    # streamed per output chunk (twice); LN of half h overlaps proj of h+1
    y_q = big.tile([128, NQF, S], BF, tag="B")        # q latent pre-LN
    y_kv = big.tile([128, NKVF, S], BF, tag="C")      # kv latent pre-LN + rope
    SH = S // 2

    def layernorm_sc(y, nf, F, w_pp, b_pp, sc):
        ssl = slice(sc * SC, (sc + 1) * SC)
        s1 = ps_acc.tile([1, SC], F32, tag="accl")
        s2 = ps_acc.tile([1, SC], F32, tag="accl")
        for f in range(nf):
            y2 = b512.tile([128, SC], BF, tag="b512")
            nc.scalar.square(y2[:], y[:, f, ssl])
            nc.tensor.matmul(s1[:], ones_bf[:], y[:, f, ssl],
                             start=(f == 0), stop=(f == nf - 1))
            nc.tensor.matmul(s2[:], ones_bf[:], y2[:],
                             start=(f == 0), stop=(f == nf - 1))
        # E[y] and E[y^2] as fp32r rows, broadcast wide via K=1 matmul
        mu_r = vec1.tile([1, SC], F32R, tag="vec1r")
        ms_r = vec1.tile([1, SC], F32R, tag="vec1r")
        nc.scalar.activation(mu_r[:], s1[:],
                             mybir.ActivationFunctionType.Copy, scale=1.0 / F)
        nc.scalar.activation(ms_r[:], s2[:],
                             mybir.ActivationFunctionType.Copy, scale=1.0 / F)
        mub_ps = ps_mm.tile([128, SC], F32, tag="mm")
        msb_ps = ps_mm.tile([128, SC], F32, tag="mm")
        nc.tensor.matmul(mub_ps[:], ones_f1[:], mu_r[:], start=True, stop=True)
        nc.tensor.matmul(msb_ps[:], ones_f1[:], ms_r[:], start=True, stop=True)
        # wide finalize: rs = 1/sqrt(E[y^2] - mu^2 + eps) on 128 lanes
        mub_sb = b512.tile([128, SC], BF, tag="b512")
        nc.scalar.copy(mub_sb[:], mub_ps[:])
        var_sb = b512.tile([128, SC], F32, tag="b512")
        nc.vector.tensor_tensor(var_sb[:], mub_ps[:], mub_sb[:],
                                mybir.AluOpType.mult)
        nc.vector.tensor_tensor(var_sb[:], msb_ps[:], var_sb[:],
                                mybir.AluOpType.subtract)
        nc.scalar.activation(var_sb[:], var_sb[:],
                             mybir.ActivationFunctionType.Sqrt,
                             bias=eps128[:])
        rsf_sb = b512.tile([128, SC], F32, tag="b512")
        nc.vector.reciprocal_approx_fast(rsf_sb[:], var_sb[:])
        rsb_sb = b512.tile([128, SC], BF, tag="b512")
        nc.scalar.copy(rsb_sb[:], rsf_sb[:])
        # apply in place: y = (y - mu)*rs*w + b
        for f in range(nf):
            nc.vector.tensor_tensor(y[:, f, ssl], y[:, f, ssl], mub_sb[:],
                                    mybir.AluOpType.subtract)
            nc.vector.tensor_tensor(y[:, f, ssl], y[:, f, ssl], rsb_sb[:],
                                    mybir.AluOpType.mult)
            nc.vector.tensor_scalar(y[:, f, ssl], y[:, f, ssl],
                                    w_pp[:, f:f + 1], b_pp[:, f:f + 1],
                                    mybir.AluOpType.mult,
                                    mybir.AluOpType.add)

    for half in range(2):
        x_sb = big.tile([128, NKO, SH], BF, tag="A")
        nc.sync.dma_start(x_sb[:, :, :SH // 2],
                          xT[:, :, half * SH:half * SH + SH // 2])
        nc.sync.dma_start(x_sb[:, :, SH // 2:],
                          xT[:, :, half * SH + SH // 2:(half + 1) * SH])
        for dst, wap, nf in ((y_q, qaw, NQF), (y_kv, kvaw, NKVF)):
            for f in range(nf):
                wt = wstream.tile([128, NKO, 128], BF, tag="w")
                nc.sync.dma_start(wt[:], wap[:, :, f * 128:(f + 1) * 128])
                nsch = SH // SC
                pss = [ps_mm.tile([128, SC], F32, tag="mm", name=f"psA{i}")
                       for i in range(nsch)]
                for ko in range(NKO):  # weight-stationary over S-chunks
                    for sch in range(nsch):
                        nc.tensor.matmul(pss[sch][:], wt[:, ko, :],
                                         x_sb[:, ko, sch * SC:(sch + 1) * SC],
                                         start=(ko == 0), stop=(ko == NKO - 1))
                for sch in range(nsch):
                    sc = half * nsch + sch
                    nc.scalar.copy(dst[:, f, sc * SC:(sc + 1) * SC],
                                   pss[sch][:])
        # LN for this half's S-chunks (overlaps next half's projections)
        for sch in range(SH // SC):
            sc = half * (SH // SC) + sch
            layernorm_sc(y_q, NQF, float(Q_LORA), qlnw, qlnb, sc)
            layernorm_sc(y_kv, 4, float(KV_LORA), kvlnw, kvlnb, sc)

    # ---------------- phase B: q_b projection + RoPE (weights streamed),
    # S-halves pipelined behind the LN of phase A
    q_all = big.tile([128, NQB, S], BF, tag="A")
    for shalf in range(2):
        for m in range(NQB):
            wt = wstream.tile([128, NQF, 128], BF, tag="w")
            nc.sync.dma_start(wt[:], qbw[:, :, m * 128:(m + 1) * 128])
            pss = [ps_mm.tile([128, SC], F32, tag="mm", name=f"psB{i}")
                   for i in range(2)]
            for kc in range(NQF):  # weight-stationary over S-chunks
                for sch in range(2):
                    sc = shalf * 2 + sch
                    nc.tensor.matmul(pss[sch][:], wt[:, kc, :],
                                     y_q[:, kc, sc * SC:(sc + 1) * SC],
                                     start=(kc == 0), stop=(kc == NQF - 1))
            for sch in range(2):
                sc = shalf * 2 + sch
                nc.vector.tensor_copy(q_all[:, m, sc * SC:(sc + 1) * SC],
                                      pss[sch][:])

    # q rope: chunks 4,5 <- chunk{4,5}*cos2 + chunk{6,7}*sin2
    for rc in (4, 5):
        nc.vector.tensor_tensor(q_all[:, rc, :], q_all[:, rc, :], cos2[:],
                                mybir.AluOpType.mult)
        nc.vector.tensor_tensor(q_all[:, rc + 2, :], q_all[:, rc + 2, :],
                                sin2[:], mybir.AluOpType.mult)
        nc.vector.tensor_tensor(q_all[:, rc, :], q_all[:, rc, :],
                                q_all[:, rc + 2, :], mybir.AluOpType.add)
    # k rope: chunk4 = [rope|rope dup], chunk5 = [rot|rot dup] ->
    # chunk4 <- chunk4*cos2 + chunk5*sin2 (both partition halves at once)
    nc.vector.tensor_tensor(y_kv[:, 4, :], y_kv[:, 4, :], cos2[:],
                            mybir.AluOpType.mult)
    nc.vector.tensor_tensor(y_kv[:, 5, :], y_kv[:, 5, :], sin2[:],
                            mybir.AluOpType.mult)
    nc.vector.tensor_tensor(y_kv[:, 4, :], y_kv[:, 4, :],
                            y_kv[:, 5, :], mybir.AluOpType.add)

    # ---------------- phase C: kv_b decompress + attention
    kvbw_sb = big.tile([128, 4, 1024], BF, tag="F")
    nc.sync.dma_start(kvbw_sb[:], kvbw[:])

    # v for all heads, token-major: [128 tok, 16 tchunk, 512(4h x 128)]
    v_sb = big.tile([128, NKB, HPC * V_HD], BF, tag="D")
    for t in range(NKB):
        ps = ps_mm.tile([128, SC], F32, tag="mm")
        for kc in range(4):
            nc.tensor.matmul(ps[:], y_kv[:, kc, t * 128:(t + 1) * 128],
                             kvbw_sb[:, kc, 512:1024],
                             start=(kc == 0), stop=(kc == 3))
        nc.vector.tensor_copy(v_sb[:, t, :], ps[:])

    att_out = big.tile([128, HPC, S], BF, tag="E")
    for h in range(HPC):
        knope = kp.tile([128, S], BF, tag="knope")
        pss = [ps_mm.tile([128, SC], F32, tag="mm", name=f"psK{i}") for i in range(NSC)]
        for kc in range(4):  # weight-stationary over S-chunks
            for sc in range(NSC):
                nc.tensor.matmul(pss[sc][:], kvbw_sb[:, kc, h * 128:(h + 1) * 128],
                                 y_kv[:, kc, sc * SC:(sc + 1) * SC],
                                 start=(kc == 0), stop=(kc == 3))
        for sc in range(NSC):
            nc.vector.tensor_copy(knope[:, sc * SC:(sc + 1) * SC], pss[sc][:])

        q_nope = q_all[:, h, :]
        rp = (h % 2) * 64
        q_rope = q_all[rp:rp + 64, 4 + h // 2, :]
        for j in range(NSC):
            qsl = slice(j * SC, (j + 1) * SC)
            out_ps = ps_acc.tile([128, SC], F32, tag="accl")
            l_ps = ps_acc.tile([1, SC], F32, tag="accl")
            nkb = QPB * (j + 1)
            for kb in range(nkb):
                s_ps = ps_mm.tile([128, SC], F32, tag="mm")
                nc.tensor.matmul(s_ps[:], knope[:, kb * 128:(kb + 1) * 128],
                                 q_nope[:, qsl], start=True, stop=False)
                nc.tensor.matmul(s_ps[:], y_kv[rp:rp + 64, 4,
                                               kb * 128:(kb + 1) * 128],
                                 q_rope[:, qsl], start=False, stop=True)
                pt = ptp.tile([128, SC], BF, tag="pt")
                if kb >= QPB * j:
                    nc.vector.scalar_tensor_tensor(
                        s_ps[:], s_ps[:], SCALE, mask[:, kb - QPB * j, :],
                        mybir.AluOpType.mult, mybir.AluOpType.add)
                    nc.scalar.activation(pt[:], s_ps[:],
                                         mybir.ActivationFunctionType.Exp)
                else:
                    nc.scalar.activation(pt[:], s_ps[:],
                                         mybir.ActivationFunctionType.Exp,
                                         scale=SCALE)
                nc.tensor.matmul(out_ps[:], v_sb[:, kb, h * V_HD:(h + 1) * V_HD],
                                 pt[:], start=(kb == 0), stop=(kb == nkb - 1))
                nc.tensor.matmul(l_ps[:], ones_bf[:], pt[:],
                                 start=(kb == 0), stop=(kb == nkb - 1))
            l_r = vec1.tile([1, SC], F32R, tag="vec1r")
            nc.scalar.copy(l_r[:], l_ps[:])
            lbc = ps_mm.tile([128, SC], F32, tag="mm")
            nc.tensor.matmul(lbc[:], ones_f1[:], l_r[:], start=True, stop=True)
            linvf = b512.tile([128, SC], F32, tag="b512")
            nc.vector.reciprocal_approx_fast(linvf[:], lbc[:])
            linvb = b512.tile([128, SC], BF, tag="b512")
            nc.scalar.copy(linvb[:], linvf[:])
            nc.vector.tensor_tensor(att_out[:, h, qsl], out_ps[:], linvb[:],
                                    mybir.AluOpType.mult)

    # ---------------- phase D: o_proj partial, feature-major out
    ow_sb = big.tile([128, HPC, D_MODEL], BF, tag="B")
    nc.sync.dma_start(ow_sb[:], oww[:])
    for dc in range(D_MODEL // 128):
        pss = [ps_mm.tile([128, SC], F32, tag="mm", name=f"psD{i}") for i in range(NSC)]
        for hc in range(HPC):  # weight-stationary over S-chunks
            for sc in range(NSC):
                nc.tensor.matmul(pss[sc][:],
                                 ow_sb[:, hc, dc * 128:(dc + 1) * 128],
                                 att_out[:, hc, sc * SC:(sc + 1) * SC],
                                 start=(hc == 0), stop=(hc == HPC - 1))
        for sc in range(NSC):
            ostage = b512.tile([128, SC], F32, tag="b512")
            nc.scalar.copy(ostage[:], pss[sc][:])
            nc.sync.dma_start(outT[dc * 128:(dc + 1) * 128,
                                   sc * SC:(sc + 1) * SC], ostage[:])


def build_nc(num_devices=8):
    nc = bacc.Bacc("TRN2", target_bir_lowering=False, debug=False,
                   num_devices=num_devices)
    T = {}
    def inp(name, shape, dt):
        T[name] = nc.dram_tensor(name, shape, dt, kind="ExternalInput")
    inp("xT", [D_MODEL, S], BF)
    inp("qaw", [D_MODEL, Q_LORA], BF)
    inp("kvaw", [D_MODEL, KVE], BF)
    inp("qbw", [Q_LORA, QBO], BF)
    inp("kvbw", [KV_LORA, 1024], BF)
    inp("ow", [HPC * V_HD, D_MODEL], BF)
    inp("cos2", [128, S], BF)
    inp("sin2", [128, S], BF)
    inp("qlnw", [128, NQF], F32)
    inp("qlnb", [128, NQF], F32)
    inp("kvlnw", [128, 4], F32)
    inp("kvlnb", [128, 4], F32)
    inp("mask", [128, QPB, SC], BF)
    T["outT"] = nc.dram_tensor("outT", [D_MODEL, S], F32, kind="ExternalOutput")

    from contextlib import ExitStack
    with tile.TileContext(nc) as tc, ExitStack() as ctx:
        _emit(ctx, tc, T)
    nc.compile()
    return nc


_NC_CACHE = {}


def kernel(**inputs) -> np.ndarray:
    in_maps = _host_prep(**inputs)
    if "nc" not in _NC_CACHE:
        _NC_CACHE["nc"] = build_nc()
    nc = _NC_CACHE["nc"]
    res = run_bass_kernel_spmd(nc, in_maps, core_ids=list(range(8)))
    out = np.zeros((B, S, D_MODEL), np.float32)
    for c in range(8):
        out[c // 4] += res.results[c]["outT"].T
    return out
